# revision 2
# baseline (speedup 1.0000x reference)
"""Trainium2 Bass kernel for speaker-rate positional-encoding attention.

Folded formulation — the host precomputes weight products and the (input-
dependent) positional encodings; the device runs a pure GEMM+softmax
pipeline at the PE roofline:
  A   = Wq^T @ Wk              scores = Xq_full @ A @ Xk_full^T
  Wvo = Wo @ Wv                vpp    = values @ Wvo^T
  bo2 = bo + (Wo @ bv)/sqrt(D)     (attn rows sum to 1 so bv folds out)
  Xq_full = query + pe_q, Xk_full = keys + pe_k   (host, numpy sin)
The bk-term of scores is constant per t-row => softmax-invariant => dropped.
The bq-term varies per s-row => applied as a per-partition bias in the exp.

Per batch on device (T-layout, features on partition; all GEMM operands
bf16, accumulation f32 in PSUM):
  kAT  = A-proj of kfullT             [e-part, s]    64 MM
  vppT = values-proj with Wvo         [s-part, e]    64 MM
  Et   = exp(kAT^T qfullT / sqrt(D))  [s-part, t]    64 MM
  out  = (Et^T vppT) * recip + bo2    [t-part, e]    64 MM + 32 colsum MM

8 NeuronCores, data-parallel over batch (2 per core), no collectives.
"""

import sys

for _p in ("/opt/trn_rl_repo",):
    if _p not in sys.path:
        sys.path.insert(0, _p)

import numpy as np
import ml_dtypes

import concourse.bass as bass
from concourse import bacc
import concourse.mybir as mybir
import concourse.tile as tile
from concourse.bass_utils import run_bass_kernel_spmd

B, T, S, D, SPK = 16, 1024, 512, 1024, 256
NCORES = 8
BL = B // NCORES          # batches per core
P = 128
DT = D // P               # 8 d-tiles (also e-tiles)
ST = S // P               # 4 s-tiles
TT = T // P               # 8 t-tiles
F32 = mybir.dt.float32
BF16 = mybir.dt.bfloat16
NPBF16 = ml_dtypes.bfloat16
SQRT_D = float(np.sqrt(D))

AF = mybir.ActivationFunctionType
ALU = mybir.AluOpType


def build_nc():
    nc = bacc.Bacc()
    dp = nc.declare_dram_parameter
    qTi = dp("qTi", [BL, P, DT, T], BF16, isOutput=False)   # (query+pe)^T
    kTi = dp("kTi", [BL, P, DT, S], BF16, isOutput=False)   # (keys+pe)^T
    vTi = dp("vTi", [BL, P, DT, S], BF16, isOutput=False)   # values^T
    wa = dp("wa", [P, DT, D], BF16, isOutput=False)         # A^T  [p_d,dt,e]
    wvo = dp("wvo", [P, DT, D], BF16, isOutput=False)       # Wvo^T [p_d,dt,e]
    cb = dp("cb", [P, BL, ST], F32, isOutput=False)         # bq-term / sqrt(D)
    bo2 = dp("bo2", [D], F32, isOutput=False)
    out = dp("out", [BL, TT, P, D], BF16, isOutput=True)

    def bcast(ap, n=P):
        # replicate a DRAM vector across n partitions (DMA partition-step 0)
        return bass.AP(tensor=ap.tensor, offset=ap.offset, ap=[[0, n], *ap.ap])

    with tile.TileContext(nc) as tc:
        with (
            tc.tile_pool(name="consts", bufs=1) as cp,
            tc.tile_pool(name="big", bufs=2) as bp,        # per-batch acts
            tc.tile_pool(name="outp", bufs=4) as op,
            tc.tile_pool(name="psum", bufs=8, space="PSUM") as pp,
        ):
            # ---------------- constants ----------------
            # weights + small consts ride the gpsimd DMA queue so the sync
            # queue is free for the per-batch input loads from t=0
            cb_sb = cp.tile([P, BL, ST], F32)
            nc.gpsimd.dma_start(out=cb_sb, in_=cb[:])
            bo2_bc = cp.tile([P, D], F32)
            nc.gpsimd.dma_start(out=bo2_bc, in_=bcast(bo2[:]))
            wvo_sb = cp.tile([P, DT, D], BF16)
            for ec in range(2):
                nc.gpsimd.dma_start(out=wvo_sb[:, :, ec * 512:(ec + 1) * 512],
                                    in_=wvo[:, :, ec * 512:(ec + 1) * 512])
            wa_sb = cp.tile([P, DT, D], BF16)
            nc.gpsimd.dma_start(out=wa_sb, in_=wa[:])
            ones_sb = cp.tile([P, 2], BF16)
            nc.vector.memset(ones_sb, SQRT_D)   # folds the out/sqrt(D) scale

            # ---------------- per-batch pipeline ----------------
            for b in range(BL):
                kfullT = bp.tile([P, DT, S], BF16, tag="kfullT")
                qfullT = bp.tile([P, DT, T], BF16, tag="qfullT")
                vT = bp.tile([P, DT, S], BF16, tag="vT")
                kAT = bp.tile([P, DT, S], BF16, tag="kAT")
                vppT = bp.tile([P, ST, D], BF16, tag="vppT")
                Et = bp.tile([P, ST, T], BF16, tag="Et")
                recip = bp.tile([P, TT], F32, tag="recip")

                nc.sync.dma_start(out=vT, in_=vTi[b])
                nc.sync.dma_start(out=kfullT, in_=kTi[b])
                nc.sync.dma_start(out=qfullT, in_=qTi[b])

                # ---- vppT[s-part, e] = values-row-proj (accumulate over dt)
                _sid_vpp, _ = nc.enter_named_scope(f"b{b}_vpp", False)
                for ec in range(2):
                    for st in range(ST):
                        ps = pp.tile([P, 512], F32, tag="ps")
                        for dt in range(DT):
                            nc.tensor.matmul(
                                ps,
                                vT[:, dt, st * P:(st + 1) * P],
                                wvo_sb[:, dt, ec * 512:(ec + 1) * 512],
                                start=(dt == 0), stop=(dt == DT - 1),
                            )
                        nc.scalar.activation(
                            vppT[:, st, ec * 512:(ec + 1) * 512], ps,
                            AF.Copy, bias=0.0, scale=1.0)

                # ---- kAT[e-part, s] = A-proj of kfullT (accumulate over dt)
                nc.leave_named_scope(f"b{b}_vpp", _sid_vpp, False)
                _sid_kA, _ = nc.enter_named_scope(f"b{b}_kA", False)
                for et in range(DT):
                    ps = pp.tile([P, 512], F32, tag="ps")
                    for dt in range(DT):
                        nc.tensor.matmul(
                            ps,
                            wa_sb[:, dt, et * P:(et + 1) * P],
                            kfullT[:, dt, :],
                            start=(dt == 0), stop=(dt == DT - 1),
                        )
                    nc.scalar.activation(
                        kAT[:, et, :], ps, AF.Copy, bias=0.0, scale=1.0)

                # ---- Et[s-part, t] = exp((kAT^T @ qfullT + bq-term)/sqrt(D))
                nc.leave_named_scope(f"b{b}_kA", _sid_kA, False)
                _sid_Et, _ = nc.enter_named_scope(f"b{b}_Et", False)
                for st in range(ST):
                    for tc_ in range(2):
                        ps = pp.tile([P, 512], F32, tag="ps")
                        for et in range(DT):
                            nc.tensor.matmul(
                                ps,
                                kAT[:, et, st * P:(st + 1) * P],
                                qfullT[:, et, tc_ * 512:(tc_ + 1) * 512],
                                start=(et == 0), stop=(et == DT - 1),
                            )
                        nc.scalar.activation(
                            Et[:, st, tc_ * 512:(tc_ + 1) * 512], ps,
                            AF.Exp, bias=cb_sb[:, b, st:st + 1],
                            scale=1.0 / SQRT_D)

                nc.leave_named_scope(f"b{b}_Et", _sid_Et, False)
                _sid_fin, _ = nc.enter_named_scope(f"b{b}_fin", False)
                # ---- recip[t] = 1 / (sqrt(D) * sum_s Et[s,t])
                for tt in range(TT):
                    ps = pp.tile([P, 512], F32, tag="ps")
                    for st in range(ST):
                        nc.tensor.matmul(
                            ps[:, :2],
                            Et[:, st, tt * P:(tt + 1) * P],
                            ones_sb,
                            start=(st == 0), stop=(st == ST - 1),
                        )
                    nc.vector.reciprocal(recip[:, tt:tt + 1], ps[:, 0:1])

                # ---- out[t-part, e] = recip * (Et^T @ vppT) + bo2
                for tt in range(TT):
                    osb = op.tile([P, D], BF16, tag="osb")
                    for ec in range(2):
                        ps = pp.tile([P, 512], F32, tag="ps")
                        for st in range(ST):
                            nc.tensor.matmul(
                                ps,
                                Et[:, st, tt * P:(tt + 1) * P],
                                vppT[:, st, ec * 512:(ec + 1) * 512],
                                start=(st == 0), stop=(st == ST - 1),
                            )
                        nc.vector.scalar_tensor_tensor(
                            osb[:, ec * 512:(ec + 1) * 512],
                            ps, recip[:, tt:tt + 1],
                            bo2_bc[:, ec * 512:(ec + 1) * 512],
                            ALU.mult, ALU.add)
                    nc.scalar.dma_start(out=out[b, tt], in_=osb)
                nc.leave_named_scope(f"b{b}_fin", _sid_fin, False)
    return nc


def _pos_enc_host(length, rate, pos0):
    # pe[b, l, d] = sin/cos(rate_b * (pos0 + l) * 10000^(-d/D))
    pos = (np.arange(length, dtype=np.float64) + pos0)
    dvec = np.arange(D, dtype=np.float64)
    invdiv = 10000.0 ** (-dvec / D)
    ang = rate.astype(np.float64)[:, None, None] * pos[None, :, None] \
        * invdiv[None, None, :]
    even = (np.arange(D) % 2) == 0
    return np.where(even[None, None, :], np.sin(ang), np.cos(ang)).astype(
        np.float32)


def marshal_inputs(query, keys, values, speaker_embedding, Wsq, bsq, Wsk, bsk,
                   Wq, bq, Wk, bk, Wv, bv, Wo, bo, current_mel_pos):
    f = lambda x: np.asarray(x, dtype=np.float32)
    query, keys, values = f(query), f(keys), f(values)
    spk = f(speaker_embedding)
    Wsq, Wsk = f(Wsq), f(Wsk)
    Wq, Wk, Wv, Wo = f(Wq), f(Wk), f(Wv), f(Wo)
    bq, bk, bv, bo = f(bq), f(bk), f(bv), f(bo)
    bsq, bsk = f(bsq), f(bsk)
    mel0 = int(np.asarray(current_mel_pos).item())

    # host-side weight folding (f64 for the products)
    A = (Wq.astype(np.float64).T @ Wk.astype(np.float64)).astype(np.float32)
    Wvo = (Wo.astype(np.float64) @ Wv.astype(np.float64)).astype(np.float32)
    bo2 = bo + (Wo @ bv) / SQRT_D

    # speaker-dependent position rates + positional encodings
    sig = lambda x: 1.0 / (1.0 + np.exp(-x))
    rate_q = sig(spk @ Wsq.T + bsq)[:, 0]   # (B,)
    rate_k = sig(spk @ Wsk.T + bsk)[:, 0]
    q_full = query + _pos_enc_host(T, rate_q, mel0)
    k_full = keys + _pos_enc_host(S, rate_k, 0)

    # bq-term of scores: c[s] = bq . (k_full[s] @ Wk^T + bk); per-s softmax
    # bias. Zero in practice (bq == 0), skip the matmul then.
    if np.any(bq):
        cvec = ((k_full @ Wk.T + bk) @ bq).astype(np.float32) / SQRT_D
    else:
        cvec = np.zeros((B, S), np.float32)

    shared = {
        "wa": np.ascontiguousarray(
            A.T.reshape(DT, P, D).transpose(1, 0, 2)).astype(NPBF16),
        "wvo": np.ascontiguousarray(
            Wvo.T.reshape(DT, P, D).transpose(1, 0, 2)).astype(NPBF16),
        "bo2": bo2,
    }
    # [b, l, d] -> [b, p_d, dt, l]  (features on partition, d = dt*P + p)
    tr = lambda x, L: np.ascontiguousarray(
        x.reshape(BL, L, DT, P).transpose(0, 3, 2, 1))
    in_maps = []
    for c in range(NCORES):
        sl = slice(c * BL, (c + 1) * BL)
        m = dict(shared)
        m["qTi"] = tr(q_full[sl].astype(NPBF16), T)
        m["kTi"] = tr(k_full[sl].astype(NPBF16), S)
        m["vTi"] = tr(values[sl].astype(NPBF16), S)
        # cb[p, b, st] = cvec[b, st*P + p]
        m["cb"] = np.ascontiguousarray(
            cvec[sl].reshape(BL, ST, P).transpose(2, 0, 1))
        in_maps.append(m)
    return in_maps


def run_device(in_maps, trace=False, **kw):
    nc = build_nc()
    if not nc.is_finalized():
        nc.finalize()
    res = run_bass_kernel_spmd(nc, in_maps, core_ids=list(range(NCORES)),
                               trace=trace, **kw)
    outs = [np.asarray(r["out"]).astype(np.float32).reshape(BL, T, D)
            for r in res.results]
    return np.concatenate(outs, axis=0), res


def kernel(**inputs) -> np.ndarray:
    in_maps = marshal_inputs(**inputs)
    out, _ = run_device(in_maps)
    return out


# revision 3
# speedup vs baseline: 1.1970x; 1.1970x over previous
"""Trainium2 Bass kernel for speaker-rate positional-encoding attention.

Folded formulation — the host precomputes weight products and the (input-
dependent) positional encodings; the device runs a pure GEMM+softmax
pipeline at the PE roofline:
  A   = Wq^T @ Wk              scores = Xq_full @ A @ Xk_full^T
  Wvo = Wo @ Wv                vpp    = values @ Wvo^T
  bo2 = bo + (Wo @ bv)/sqrt(D)     (attn rows sum to 1 so bv folds out)
  Xq_full = query + pe_q, Xk_full = keys + pe_k   (host, numpy sin)
The bk-term of scores is constant per t-row => softmax-invariant => dropped.
The bq-term varies per s-row => applied as a per-partition bias in the exp.

Per batch on device (T-layout, features on partition; all GEMM operands
bf16, accumulation f32 in PSUM):
  kAT  = A-proj of kfullT             [e-part, s]    64 MM
  vppT = values-proj with Wvo         [s-part, e]    64 MM
  Et   = exp(kAT^T qfullT / sqrt(D))  [s-part, t]    64 MM
  out  = (Et^T vppT) * recip + bo2    [t-part, e]    64 MM + 32 colsum MM

8 NeuronCores, data-parallel over batch (2 per core), no collectives.
"""

import sys

for _p in ("/opt/trn_rl_repo",):
    if _p not in sys.path:
        sys.path.insert(0, _p)

import numpy as np
import ml_dtypes

import concourse.bass as bass
from concourse import bacc
import concourse.mybir as mybir
import concourse.tile as tile
from concourse.bass_utils import run_bass_kernel_spmd

B, T, S, D, SPK = 16, 1024, 512, 1024, 256
NCORES = 8
BL = B // NCORES          # batches per core
P = 128
DT = D // P               # 8 d-tiles (also e-tiles)
ST = S // P               # 4 s-tiles
TT = T // P               # 8 t-tiles
F32 = mybir.dt.float32
BF16 = mybir.dt.bfloat16
NPBF16 = ml_dtypes.bfloat16
SQRT_D = float(np.sqrt(D))

AF = mybir.ActivationFunctionType
ALU = mybir.AluOpType


def build_nc():
    nc = bacc.Bacc()
    dp = nc.declare_dram_parameter
    qTi = dp("qTi", [BL, P, DT, T], BF16, isOutput=False)   # (query+pe)^T
    kTi = dp("kTi", [BL, P, DT, S], BF16, isOutput=False)   # (keys+pe)^T
    vTi = dp("vTi", [BL, P, DT, S], BF16, isOutput=False)   # values^T
    wa = dp("wa", [P, DT, D], BF16, isOutput=False)         # A^T  [p_d,dt,e]
    wvo = dp("wvo", [P, DT, D], BF16, isOutput=False)       # Wvo^T [p_d,dt,e]
    cb = dp("cb", [P, BL, ST], F32, isOutput=False)         # bq-term / sqrt(D)
    bo2 = dp("bo2", [D], F32, isOutput=False)
    out = dp("out", [BL, P, TT, D], BF16, isOutput=True)

    def bcast(ap, n=P):
        # replicate a DRAM vector across n partitions (DMA partition-step 0)
        return bass.AP(tensor=ap.tensor, offset=ap.offset, ap=[[0, n], *ap.ap])

    with tile.TileContext(nc) as tc:
        with (
            tc.tile_pool(name="consts", bufs=1) as cp,
            tc.tile_pool(name="big", bufs=2) as bp,        # per-batch acts
            tc.tile_pool(name="outp", bufs=4) as op,
            tc.tile_pool(name="psum", bufs=8, space="PSUM") as pp,
        ):
            # ---------------- constants ----------------
            # two fast DMA queues (sync + scalar) are interleaved per
            # dt-slab so the first vpp matmul can start ~2us in; the
            # late-needed small consts ride the slow gpsimd queue
            wvo_sb = cp.tile([P, DT, D], BF16)
            wa_sb = cp.tile([P, DT, D], BF16)
            cb_sb = cp.tile([P, BL, ST], F32)
            bo2_bc = cp.tile([P, D], F32)
            ones_sb = cp.tile([P, 2], BF16)
            nc.vector.memset(ones_sb, SQRT_D)   # folds the out/sqrt(D) scale
            # HAM warm-up: dummy matmuls fill the DMA-wait window so the
            # PE clock-gate releases (1.2->2.4 GHz) before the real stream
            warm = cp.tile([P, 512], BF16)
            nc.vector.memset(warm, 0.0)
            wps = pp.tile([P, 512], F32, tag="ps", name="wps")
            for _ in range(25):
                nc.tensor.matmul(wps, warm[:, :P], warm, start=True, stop=True)
            # ---------------- per-batch pipeline ----------------
            # phases are interleaved across the two batches so every
            # phase-boundary copy/exp tail hides under the other batch's
            # independent matmuls; within a phase, MM pairs share their
            # stationary operand so LDWEIGHTS can be amortized
            tiles = []
            for b in range(BL):
                tiles.append(dict(
                    kfullT=bp.tile([P, DT, S], BF16, tag="kfullT",
                                   name=f"kfullT{b}"),
                    qfullT=bp.tile([P, DT, T], BF16, tag="qfullT",
                                   name=f"qfullT{b}"),
                    vT=bp.tile([P, DT, S], BF16, tag="vT", name=f"vT{b}"),
                    kAT=bp.tile([P, DT, S], BF16, tag="kAT", name=f"kAT{b}"),
                    vppT=bp.tile([P, ST, D], BF16, tag="vppT",
                                 name=f"vppT{b}"),
                    Et=bp.tile([P, ST, T], BF16, tag="Et", name=f"Et{b}"),
                    recip=bp.tile([P, TT], F32, tag="recip",
                                  name=f"recip{b}"),
                ))

            def prep01():
                # two fast queues (sync/scalar; gpsimd SWDGE is ~25us+
                # latency), big DMAs in PE need-order. wvo is split across
                # both queues and vT0's st=0 slab loads first so the first
                # real matmul can start ~12us in; warm-up dummies cover
                # the wait and release the HAM clock gate.
                nc.sync.dma_start(out=wvo_sb[:, :DT // 2, :],
                                  in_=wvo[:, :DT // 2, :])
                nc.scalar.dma_start(out=wvo_sb[:, DT // 2:, :],
                                    in_=wvo[:, DT // 2:, :])
                nc.sync.dma_start(out=tiles[0]["vT"][:, :, 0:P],
                                  in_=vTi[0][:, :, 0:P])
                nc.scalar.dma_start(out=tiles[0]["vT"][:, :, P:],
                                    in_=vTi[0][:, :, P:])
                nc.sync.dma_start(out=tiles[0]["kfullT"], in_=kTi[0])
                nc.scalar.dma_start(out=wa_sb, in_=wa[:])
                nc.sync.dma_start(out=tiles[1]["vT"], in_=vTi[1])
                nc.scalar.dma_start(out=tiles[0]["qfullT"], in_=qTi[0])
                nc.sync.dma_start(out=tiles[1]["qfullT"], in_=qTi[1])
                nc.scalar.dma_start(out=tiles[1]["kfullT"], in_=kTi[1])
                nc.scalar.dma_start(out=cb_sb, in_=cb[:])
                nc.scalar.dma_start(out=bo2_bc, in_=bcast(bo2[:]))

            def vpp_phase(b):
                tl = tiles[b]
                sid, _ = nc.enter_named_scope(f"b{b}_vpp", False)
                for st in range(ST):
                    psa = pp.tile([P, 512], F32, tag="ps")
                    psb = pp.tile([P, 512], F32, tag="ps")
                    for dt in range(DT):
                        for ec, ps in ((0, psa), (1, psb)):
                            nc.tensor.matmul(
                                ps,
                                tl["vT"][:, dt, st * P:(st + 1) * P],
                                wvo_sb[:, dt, ec * 512:(ec + 1) * 512],
                                start=(dt == 0), stop=(dt == DT - 1),
                            )
                    for ec, ps in ((0, psa), (1, psb)):
                        nc.scalar.activation(
                            tl["vppT"][:, st, ec * 512:(ec + 1) * 512], ps,
                            AF.Copy, bias=0.0, scale=1.0)
                nc.leave_named_scope(f"b{b}_vpp", sid, False)

            def kA_phase(b):
                tl = tiles[b]
                sid, _ = nc.enter_named_scope(f"b{b}_kA", False)
                for et in range(DT):
                    ps = pp.tile([P, 512], F32, tag="ps")
                    for dt in range(DT):
                        nc.tensor.matmul(
                            ps,
                            wa_sb[:, dt, et * P:(et + 1) * P],
                            tl["kfullT"][:, dt, :],
                            start=(dt == 0), stop=(dt == DT - 1),
                        )
                    nc.scalar.activation(
                        tl["kAT"][:, et, :], ps, AF.Copy, bias=0.0, scale=1.0)
                nc.leave_named_scope(f"b{b}_kA", sid, False)

            def et_phase(b):
                tl = tiles[b]
                sid, _ = nc.enter_named_scope(f"b{b}_Et", False)
                for st in range(ST):
                    psa = pp.tile([P, 512], F32, tag="ps")
                    psb = pp.tile([P, 512], F32, tag="ps")
                    for et in range(DT):
                        for tc_, ps in ((0, psa), (1, psb)):
                            nc.tensor.matmul(
                                ps,
                                tl["kAT"][:, et, st * P:(st + 1) * P],
                                tl["qfullT"][:, et, tc_ * 512:(tc_ + 1) * 512],
                                start=(et == 0), stop=(et == DT - 1),
                            )
                    for tc_, ps in ((0, psa), (1, psb)):
                        nc.scalar.activation(
                            tl["Et"][:, st, tc_ * 512:(tc_ + 1) * 512], ps,
                            AF.Exp, bias=cb_sb[:, b, st:st + 1],
                            scale=1.0 / SQRT_D)
                nc.leave_named_scope(f"b{b}_Et", sid, False)

            def fin_phase(b):
                tl = tiles[b]
                sid, _ = nc.enter_named_scope(f"b{b}_fin", False)
                osb2 = None
                for tt in range(TT):
                    psc = pp.tile([P, 512], F32, tag="ps")
                    psa = pp.tile([P, 512], F32, tag="ps")
                    psb = pp.tile([P, 512], F32, tag="ps")
                    for st in range(ST):
                        stat = tl["Et"][:, st, tt * P:(tt + 1) * P]
                        nc.tensor.matmul(psc[:, :2], stat, ones_sb,
                                         start=(st == 0), stop=(st == ST - 1))
                        for ec, ps in ((0, psa), (1, psb)):
                            nc.tensor.matmul(
                                ps, stat,
                                tl["vppT"][:, st, ec * 512:(ec + 1) * 512],
                                start=(st == 0), stop=(st == ST - 1),
                            )
                    nc.vector.reciprocal(tl["recip"][:, tt:tt + 1], psc[:, 0:1])
                    if tt % 2 == 0:
                        osb2 = op.tile([P, 2, D], BF16, tag="osb",
                                       name=f"osb{b}_{tt}")
                    for ec, ps in ((0, psa), (1, psb)):
                        nc.vector.scalar_tensor_tensor(
                            osb2[:, tt % 2, ec * 512:(ec + 1) * 512],
                            ps, tl["recip"][:, tt:tt + 1],
                            bo2_bc[:, ec * 512:(ec + 1) * 512],
                            ALU.mult, ALU.add)
                    if tt >= 6:
                        eng = nc.sync if tt % 2 == 0 else nc.scalar
                        eng.dma_start(out=out[b, :, tt:tt + 1, :],
                                      in_=osb2[:, tt % 2:tt % 2 + 1, :])
                    elif tt % 2 == 1:
                        eng = nc.sync if tt % 4 == 1 else nc.scalar
                        eng.dma_start(out=out[b, :, tt - 1:tt + 1, :],
                                      in_=osb2)
                nc.leave_named_scope(f"b{b}_fin", sid, False)

            prep01()
            vpp_phase(0)
            kA_phase(0)
            vpp_phase(1)
            et_phase(0)
            kA_phase(1)
            fin_phase(0)
            et_phase(1)
            fin_phase(1)
    return nc


def _pos_enc_host(length, rate, pos0):
    # pe[b, l, d] = sin/cos(rate_b * (pos0 + l) * 10000^(-d/D))
    pos = (np.arange(length, dtype=np.float64) + pos0)
    dvec = np.arange(D, dtype=np.float64)
    invdiv = 10000.0 ** (-dvec / D)
    ang = rate.astype(np.float64)[:, None, None] * pos[None, :, None] \
        * invdiv[None, None, :]
    even = (np.arange(D) % 2) == 0
    return np.where(even[None, None, :], np.sin(ang), np.cos(ang)).astype(
        np.float32)


def marshal_inputs(query, keys, values, speaker_embedding, Wsq, bsq, Wsk, bsk,
                   Wq, bq, Wk, bk, Wv, bv, Wo, bo, current_mel_pos):
    f = lambda x: np.asarray(x, dtype=np.float32)
    query, keys, values = f(query), f(keys), f(values)
    spk = f(speaker_embedding)
    Wsq, Wsk = f(Wsq), f(Wsk)
    Wq, Wk, Wv, Wo = f(Wq), f(Wk), f(Wv), f(Wo)
    bq, bk, bv, bo = f(bq), f(bk), f(bv), f(bo)
    bsq, bsk = f(bsq), f(bsk)
    mel0 = int(np.asarray(current_mel_pos).item())

    # host-side weight folding (f64 for the products)
    A = (Wq.astype(np.float64).T @ Wk.astype(np.float64)).astype(np.float32)
    Wvo = (Wo.astype(np.float64) @ Wv.astype(np.float64)).astype(np.float32)
    bo2 = bo + (Wo @ bv) / SQRT_D

    # speaker-dependent position rates + positional encodings
    sig = lambda x: 1.0 / (1.0 + np.exp(-x))
    rate_q = sig(spk @ Wsq.T + bsq)[:, 0]   # (B,)
    rate_k = sig(spk @ Wsk.T + bsk)[:, 0]
    q_full = query + _pos_enc_host(T, rate_q, mel0)
    k_full = keys + _pos_enc_host(S, rate_k, 0)

    # bq-term of scores: c[s] = bq . (k_full[s] @ Wk^T + bk); per-s softmax
    # bias. Zero in practice (bq == 0), skip the matmul then.
    if np.any(bq):
        cvec = ((k_full @ Wk.T + bk) @ bq).astype(np.float32) / SQRT_D
    else:
        cvec = np.zeros((B, S), np.float32)

    shared = {
        "wa": np.ascontiguousarray(
            A.T.reshape(DT, P, D).transpose(1, 0, 2)).astype(NPBF16),
        "wvo": np.ascontiguousarray(
            Wvo.T.reshape(DT, P, D).transpose(1, 0, 2)).astype(NPBF16),
        "bo2": bo2,
    }
    # [b, l, d] -> [b, p_d, dt, l]  (features on partition, d = dt*P + p)
    tr = lambda x, L: np.ascontiguousarray(
        x.reshape(BL, L, DT, P).transpose(0, 3, 2, 1))
    in_maps = []
    for c in range(NCORES):
        sl = slice(c * BL, (c + 1) * BL)
        m = dict(shared)
        m["qTi"] = tr(q_full[sl].astype(NPBF16), T)
        m["kTi"] = tr(k_full[sl].astype(NPBF16), S)
        m["vTi"] = tr(values[sl].astype(NPBF16), S)
        # cb[p, b, st] = cvec[b, st*P + p]
        m["cb"] = np.ascontiguousarray(
            cvec[sl].reshape(BL, ST, P).transpose(2, 0, 1))
        in_maps.append(m)
    return in_maps


def run_device(in_maps, trace=False, **kw):
    nc = build_nc()
    if not nc.is_finalized():
        nc.finalize()
    res = run_bass_kernel_spmd(nc, in_maps, core_ids=list(range(NCORES)),
                               trace=trace, **kw)
    outs = [np.asarray(r["out"]).astype(np.float32)
            .reshape(BL, P, TT, D).transpose(0, 2, 1, 3).reshape(BL, T, D)
            for r in res.results]
    return np.concatenate(outs, axis=0), res


def kernel(**inputs) -> np.ndarray:
    in_maps = marshal_inputs(**inputs)
    out, _ = run_device(in_maps)
    return out
